# revision 4
# baseline (speedup 1.0000x reference)
"""Trainium2 Bass kernel for nn_Deconv2DVariableWeights (8-core SPMD).

Strategy:
  Phase 1 (dense + BN), unit-sharded: core c owns dense_w columns
  [c*18432, (c+1)*18432) = conv input channels s in [16c, 16c+16).
  It computes h = relu(z @ w + b) for ALL 32 samples in (units, batch)
  layout, so BatchNorm batch statistics are local free-dim reductions.
  Normalized kernels are PE-transposed to (batch, units) rows and
  written to DRAM.

  AllToAll redistributes kernels: afterwards core c holds the FULL
  147456-long kernel vector for its 4 samples (batch-sharded).

  Phase 2 (conv + residual), batch-sharded: per sample, the 3x3
  conv is 9 PSUM-accumulated matmuls (K=s=128, M=f=128, N=512x2)
  against a zero-padded image tile addressed with strided APs.
"""

import numpy as np

import concourse.bacc as bacc
import concourse.bass as bass
import concourse.tile as tile
from concourse import mybir
from concourse.bass_utils import run_bass_kernel_spmd
from concourse.masks import make_identity

# Problem constants (hardcoded per contract)
B, C, H, W = 32, 128, 32, 32
ZDIM = 256
KH = KW = 3
NB = C * C * KH * KW          # 147456
N_CORES = 8
NB_L = NB // N_CORES          # 18432 units per core
B_L = B // N_CORES            # 4 samples per core
S_L = C // N_CORES            # 16 input channels per core
N_TILES = NB_L // 128         # 144 unit tiles per core
GROUP = 8                     # unit tiles per processing group
N_GROUPS = N_TILES // GROUP   # 18
BN_EPS = 1e-6
PIX = H * W                   # 1024
PAD = H + 2                   # 34

# Precision knobs: dtype of matmul inputs (w, z) and of the generated
# conv kernels (through the AllToAll and the conv lhsT / rhs).
W_DT = mybir.dt.bfloat16
K_DT = mybir.dt.bfloat16

_CACHE: dict = {}


def _np_of(dt):
    return mybir.dt.np(dt)


def _build_nc():
    nc = bacc.Bacc(num_devices=N_CORES)
    f32 = mybir.dt.float32

    w_in = nc.declare_dram_parameter("w", [128, 2 * NB_L], W_DT, isOutput=False)
    zT_in = nc.declare_dram_parameter("zT", [128, 2 * B], W_DT, isOutput=False)
    db_in = nc.declare_dram_parameter("db_r", [128, N_TILES], f32, isOutput=False)
    gam_in = nc.declare_dram_parameter("gamma_r", [128, N_TILES], f32, isOutput=False)
    bet_in = nc.declare_dram_parameter("beta_r", [128, N_TILES], f32, isOutput=False)
    x_in = nc.declare_dram_parameter("x", [B_L, C, PIX], f32, isOutput=False)
    bv_in = nc.declare_dram_parameter("bvec", [C, 1], f32, isOutput=False)
    out_p = nc.declare_dram_parameter("out", [B_L, C, PIX], f32, isOutput=True)

    cc_in = nc.dram_tensor("cc_in", [B, NB_L], K_DT)
    cc_out = nc.dram_tensor("cc_out", [B, NB_L], K_DT)

    with tile.TileContext(nc) as tc:
        with tc.tile_pool(name="singles", bufs=1) as singles:
            zT = singles.tile([128, 2 * B], W_DT)
            nc.sync.dma_start(out=zT[:, :], in_=zT_in[:, :])
            db = singles.tile([128, N_TILES], f32)
            nc.sync.dma_start(out=db[:, :], in_=db_in[:, :])
            gam = singles.tile([128, N_TILES], f32)
            nc.sync.dma_start(out=gam[:, :], in_=gam_in[:, :])
            bet = singles.tile([128, N_TILES], f32)
            nc.sync.dma_start(out=bet[:, :], in_=bet_in[:, :])
            bv = singles.tile([C, 1], f32)
            nc.sync.dma_start(out=bv[:, :], in_=bv_in[:, :])
            ident = singles.tile([128, 128], K_DT)
            make_identity(nc, ident)
            eps_t = singles.tile([128, 1], f32)
            nc.vector.memset(eps_t, BN_EPS)

            # ---------------- Phase 1: dense + BN + transpose ----------------
            with (
                tc.tile_pool(name="wpool", bufs=3) as wpool,
                tc.tile_pool(name="hpool", bufs=3) as hpool,
                tc.tile_pool(name="stat", bufs=3) as stat,
                tc.tile_pool(name="ph", bufs=2, space="PSUM") as psum_h,
                tc.tile_pool(name="pt", bufs=2, space="PSUM") as psum_t,
            ):
                for g in range(N_GROUPS):
                    # weights for this group, both zdim chunks: [k0 | k1]
                    wt = wpool.tile([128, 2 * GROUP * 128], W_DT, tag="wt")
                    gcols = GROUP * 128  # 1024
                    for k in range(2):
                        nc.sync.dma_start(
                            out=wt[:, k * gcols:(k + 1) * gcols],
                            in_=w_in[:, k * NB_L + g * gcols:
                                     k * NB_L + (g + 1) * gcols],
                        )
                    ps = psum_h.tile([128, GROUP * B], mybir.dt.float32, tag="ps")
                    for j in range(GROUP):
                        for k in range(2):
                            nc.tensor.matmul(
                                ps[:, j * B:(j + 1) * B],
                                wt[:, k * gcols + j * 128: k * gcols + (j + 1) * 128],
                                zT[:, k * B:(k + 1) * B],
                                start=(k == 0),
                                stop=(k == 1),
                            )
                    # relu(z@w + db)  (per-unit bias via per-partition ACT bias)
                    h = hpool.tile([128, GROUP * B], f32, tag="h")
                    for j in range(GROUP):
                        t = g * GROUP + j
                        nc.scalar.activation(
                            out=h[:, j * B:(j + 1) * B],
                            in_=ps[:, j * B:(j + 1) * B],
                            func=mybir.ActivationFunctionType.Relu,
                            bias=db[:, t:t + 1],
                            scale=1.0,
                        )
                    # batch stats per unit (free-dim reductions)
                    h3 = h.rearrange("p (t b) -> p t b", b=B)
                    h2 = hpool.tile([128, GROUP * B], f32, tag="h2")
                    nc.vector.tensor_mul(h2[:, :], h[:, :], h[:, :])
                    s1 = stat.tile([128, GROUP], f32, tag="s1")
                    nc.vector.reduce_sum(out=s1[:, :], in_=h3, axis=mybir.AxisListType.X)
                    s2 = stat.tile([128, GROUP], f32, tag="s2")
                    nc.vector.reduce_sum(
                        out=s2[:, :], in_=h2.rearrange("p (t b) -> p t b", b=B),
                        axis=mybir.AxisListType.X)
                    mean = stat.tile([128, GROUP], f32, tag="mean")
                    nc.scalar.mul(out=mean[:, :], in_=s1[:, :], mul=1.0 / B)
                    m2 = stat.tile([128, GROUP], f32, tag="m2")
                    nc.scalar.mul(out=m2[:, :], in_=s2[:, :], mul=1.0 / B)
                    var = stat.tile([128, GROUP], f32, tag="var")
                    nc.vector.tensor_mul(var[:, :], mean[:, :], mean[:, :])
                    nc.vector.tensor_tensor(
                        out=var[:, :], in0=m2[:, :], in1=var[:, :],
                        op=mybir.AluOpType.subtract)
                    # inv = 1/(sqrt(var) + eps); scale = gamma*inv
                    nc.scalar.sqrt(out=var[:, :], in_=var[:, :])
                    nc.scalar.add(out=var[:, :], in_=var[:, :], add=eps_t[:, 0:1])
                    inv = stat.tile([128, GROUP], f32, tag="inv")
                    nc.vector.reciprocal(out=inv[:, :], in_=var[:, :])
                    scl = stat.tile([128, GROUP], f32, tag="scl")
                    nc.vector.tensor_mul(
                        scl[:, :], inv[:, :], gam[:, g * GROUP:(g + 1) * GROUP])
                    sft = stat.tile([128, GROUP], f32, tag="sft")
                    nc.vector.tensor_mul(sft[:, :], mean[:, :], scl[:, :])
                    nc.vector.tensor_tensor(
                        out=sft[:, :], in0=bet[:, g * GROUP:(g + 1) * GROUP],
                        in1=sft[:, :], op=mybir.AluOpType.subtract)
                    # kern = h*scale + shift  (broadcast over batch), cast K_DT
                    tmp = hpool.tile([128, GROUP * B], f32, tag="tmp")
                    nc.vector.tensor_mul(
                        tmp.rearrange("p (t b) -> p t b", b=B), h3,
                        scl[:, :].to_broadcast([128, GROUP, B]))
                    kern = hpool.tile([128, GROUP * B], K_DT, tag="kern")
                    nc.vector.tensor_tensor(
                        out=kern.rearrange("p (t b) -> p t b", b=B),
                        in0=tmp.rearrange("p (t b) -> p t b", b=B),
                        in1=sft[:, :].to_broadcast([128, GROUP, B]),
                        op=mybir.AluOpType.add)
                    # transpose each (128, B) tile -> (B, 128) and store rows
                    pt = psum_t.tile([B, GROUP * 128], K_DT, tag="pt")
                    for j in range(GROUP):
                        nc.tensor.transpose(
                            pt[:, j * 128:(j + 1) * 128],
                            kern[:, j * B:(j + 1) * B],
                            ident[:, :],
                        )
                    ktr = hpool.tile([B, GROUP * 128], K_DT, tag="ktr")
                    nc.any.tensor_copy(out=ktr[:, :], in_=pt[:, :])
                    nc.sync.dma_start(
                        out=cc_in[:, g * gcols:(g + 1) * gcols],
                        in_=ktr[:, :],
                    )

            # ---------------- AllToAll: batch redistribute ----------------
            nc.gpsimd.collective_compute(
                "AllToAll",
                mybir.AluOpType.bypass,
                replica_groups=[list(range(N_CORES))],
                ins=[cc_in[:, :]],
                outs=[cc_out[:, :]],
            )

            # ---------------- Phase 2: per-sample conv + residual ----------------
            with (
                tc.tile_pool(name="conv", bufs=2) as conv,
                tc.tile_pool(name="po", bufs=2, space="PSUM") as psum_o,
            ):
                for i in range(B_L):
                    # full kernel vector for sample 4c+i: partitions = s global
                    hk = conv.tile([128, C * KH * KW], K_DT, tag="hk")
                    src = bass.AP(
                        tensor=cc_out[:, :].tensor,
                        offset=i * NB_L,
                        ap=[[B_L * NB_L, N_CORES], [C * KH * KW, S_L],
                            [1, C * KH * KW]],
                    )
                    nc.sync.dma_start(out=hk[:, :], in_=src)
                    # input image + zero-padded cast copy
                    xi = conv.tile([128, PIX], mybir.dt.float32, tag="xi")
                    nc.sync.dma_start(out=xi[:, :], in_=x_in[i, :, :])
                    xp = conv.tile([128, PAD * PAD], K_DT, tag="xp")
                    nc.vector.memset(xp[:, :], 0.0)
                    nc.vector.tensor_copy(
                        out=xp.rearrange("p (r c) -> p r c", c=PAD)[:, 1:H + 1, 1:W + 1],
                        in_=xi.rearrange("p (r c) -> p r c", c=W),
                    )
                    # 9-tap conv, PSUM accumulate; N split in 2 halves of 512
                    po = psum_o.tile([128, PIX], mybir.dt.float32, tag="po")
                    hk9 = hk.rearrange("p (f n) -> p n f", n=KH * KW)
                    xp3 = xp.rearrange("p (r c) -> p r c", c=PAD)
                    for hh in range(2):
                        for tap in range(KH * KW):
                            u, v = tap // KW, tap % KW
                            r0 = hh * 16 + u
                            nc.tensor.matmul(
                                po[:, hh * 512:(hh + 1) * 512],
                                hk9[:, tap, :],
                                xp3[:, r0:r0 + 16, v:v + W],
                                start=(tap == 0),
                                stop=(tap == KH * KW - 1),
                            )
                    # out = conv + b + x
                    ob = conv.tile([128, PIX], mybir.dt.float32, tag="ob")
                    nc.vector.tensor_add(out=ob[:, :], in0=po[:, :], in1=xi[:, :])
                    nc.scalar.add(out=ob[:, :], in_=ob[:, :], add=bv[:, 0:1])
                    nc.sync.dma_start(out=out_p[i, :, :], in_=ob[:, :])

    nc.compile()
    return nc


def _make_in_maps(x, z, dense_w, dense_b, gamma, beta, b):
    wnp = _np_of(W_DT)
    f32 = np.float32
    # zT[p, k*B + bb] = z[bb, 128k + p]
    zr = np.ascontiguousarray(z.T.astype(f32)).reshape(2, 128, B)
    zT = np.concatenate([zr[0], zr[1]], axis=1).astype(wnp)
    bvec = b.reshape(C, 1).astype(f32)
    in_maps = []
    for c in range(N_CORES):
        sl = slice(c * NB_L, (c + 1) * NB_L)
        ws = dense_w[:, sl]
        w_host = np.ascontiguousarray(
            np.concatenate([ws[:128, :], ws[128:, :]], axis=1)).astype(wnp)
        in_maps.append({
            "w": w_host,
            "zT": zT,
            "db_r": np.ascontiguousarray(
                dense_b[sl].reshape(N_TILES, 128).T).astype(f32),
            "gamma_r": np.ascontiguousarray(
                gamma[sl].reshape(N_TILES, 128).T).astype(f32),
            "beta_r": np.ascontiguousarray(
                beta[sl].reshape(N_TILES, 128).T).astype(f32),
            "x": np.ascontiguousarray(
                x[c * B_L:(c + 1) * B_L].reshape(B_L, C, PIX)).astype(f32),
            "bvec": bvec,
        })
    return in_maps


def kernel(x, z, dense_w, dense_b, gamma, beta, b):
    if "nc" not in _CACHE:
        _CACHE["nc"] = _build_nc()
    nc = _CACHE["nc"]
    in_maps = _make_in_maps(x, z, dense_w, dense_b, gamma, beta, b)
    res = run_bass_kernel_spmd(nc, in_maps, list(range(N_CORES)))
    out = np.concatenate(
        [res.results[c]["out"].reshape(B_L, C, H, W) for c in range(N_CORES)],
        axis=0,
    )
    return out.astype(np.float32)


# revision 8
# speedup vs baseline: 1.1236x; 1.1236x over previous
"""Trainium2 Bass kernel for nn_Deconv2DVariableWeights (8-core SPMD).

Strategy:
  Phase 1 (dense + BN), unit-sharded: core c owns dense_w columns
  [c*18432, (c+1)*18432) = conv input channels s in [16c, 16c+16).
  It computes h = relu(z @ w + b) for ALL 32 samples in (units, batch)
  layout, so BatchNorm batch statistics are local free-dim reductions.
  Normalized kernels are PE-transposed to (batch, units) rows and
  written to DRAM.

  AllToAll redistributes kernels: afterwards core c holds the FULL
  147456-long kernel vector for its 4 samples (batch-sharded).

  Phase 2 (conv + residual), batch-sharded: per sample, the 3x3
  conv is 9 PSUM-accumulated matmuls (K=s=128, M=f=128, N=512x2)
  against a zero-padded image tile addressed with strided APs.
"""

import numpy as np

import concourse.bacc as bacc
import concourse.bass as bass
import concourse.tile as tile
from concourse import mybir
from concourse.bass_utils import run_bass_kernel_spmd
from concourse.masks import make_identity

# Problem constants (hardcoded per contract)
B, C, H, W = 32, 128, 32, 32
ZDIM = 256
KH = KW = 3
NB = C * C * KH * KW          # 147456
N_CORES = 8
NB_L = NB // N_CORES          # 18432 units per core
B_L = B // N_CORES            # 4 samples per core
S_L = C // N_CORES            # 16 input channels per core
N_TILES = NB_L // 128         # 144 unit tiles per core
GROUP = 16                    # unit tiles per processing group
N_GROUPS = N_TILES // GROUP   # 9
BN_EPS = 1e-6
PIX = H * W                   # 1024
PAD = H + 2                   # 34

# Precision of matmul inputs (w, z) and of the generated conv kernels
# (through the AllToAll and the conv lhsT / rhs). fp16 keeps ~3.5
# significant digits vs bf16's ~2.5 at identical throughput.
W_DT = mybir.dt.float16
K_DT = mybir.dt.float16

_CACHE: dict = {}


def _np_of(dt):
    return mybir.dt.np(dt)


def _build_nc(db_zero: bool, b_zero: bool):
    nc = bacc.Bacc(num_devices=N_CORES)
    f32 = mybir.dt.float32
    GC = GROUP * 128            # columns per group in unit space (2048)
    GB = GROUP * B              # columns per group in (tile,batch) space (512)

    w_in = nc.declare_dram_parameter("w", [128, 2 * NB_L], W_DT, isOutput=False)
    zT_in = nc.declare_dram_parameter("zT", [128, 2 * B], W_DT, isOutput=False)
    db_in = nc.declare_dram_parameter("db_r", [128, N_TILES], f32, isOutput=False)
    gam_in = nc.declare_dram_parameter("gamma_r", [128, N_TILES], f32, isOutput=False)
    bet_in = nc.declare_dram_parameter("beta_r", [128, N_TILES], f32, isOutput=False)
    x_in = nc.declare_dram_parameter("x", [B_L, C, PIX], f32, isOutput=False)
    bv_in = nc.declare_dram_parameter("bvec", [C, 1], f32, isOutput=False)
    out_p = nc.declare_dram_parameter("out", [B_L, C, PIX], f32, isOutput=True)

    cc_in = nc.dram_tensor("cc_in", [B, NB_L], K_DT)
    cc_out = nc.dram_tensor("cc_out", [B, NB_L], K_DT)

    with tile.TileContext(nc) as tc:
        with tc.tile_pool(name="singles", bufs=1) as singles:
            zT = singles.tile([128, 2 * B], W_DT)
            nc.sync.dma_start(out=zT[:, :], in_=zT_in[:, :])
            gam = singles.tile([128, N_TILES], f32)
            nc.sync.dma_start(out=gam[:, :], in_=gam_in[:, :])
            bet = singles.tile([128, N_TILES], f32)
            nc.sync.dma_start(out=bet[:, :], in_=bet_in[:, :])
            if not db_zero:
                db = singles.tile([128, N_TILES], f32)
                nc.sync.dma_start(out=db[:, :], in_=db_in[:, :])
            if not b_zero:
                bv = singles.tile([C, 1], f32)
                nc.sync.dma_start(out=bv[:, :], in_=bv_in[:, :])
            ident = singles.tile([128, 128], K_DT)
            make_identity(nc, ident)

            h_all = singles.tile([128, N_TILES * B], f32)   # 18KB/partition
            s1a = singles.tile([128, N_TILES], f32)
            s2a = singles.tile([128, N_TILES], f32)
            scl = singles.tile([128, N_TILES], f32)
            sft = singles.tile([128, N_TILES], f32)

            # ---------------- Phase 1a: dense matmuls + relu + sums ----------
            with (
                tc.tile_pool(name="wpool", bufs=3) as wpool,
                tc.tile_pool(name="ph", bufs=2, space="PSUM") as psum_h,
            ):
                w3 = w_in.rearrange("p (k j) -> p k j", k=2)
                for g in range(N_GROUPS):
                    wt = wpool.tile([128, 2, GC], W_DT, tag="wt")
                    nc.sync.dma_start(
                        out=wt[:, :, :], in_=w3[:, :, g * GC:(g + 1) * GC])
                    ps = psum_h.tile([128, GB], f32, tag="ps")
                    for j in range(GROUP):
                        for k in range(2):
                            nc.tensor.matmul(
                                ps[:, j * B:(j + 1) * B],
                                wt[:, k, j * 128:(j + 1) * 128],
                                zT[:, k * B:(k + 1) * B],
                                start=(k == 0),
                                stop=(k == 1),
                            )
                    if not db_zero:
                        nc.vector.tensor_tensor(
                            out=ps.rearrange("p (t b) -> p t b", b=B),
                            in0=ps.rearrange("p (t b) -> p t b", b=B),
                            in1=db[:, g * GROUP:(g + 1) * GROUP]
                            .to_broadcast([128, GROUP, B]),
                            op=mybir.AluOpType.add)
                    hg = h_all[:, g * GB:(g + 1) * GB]
                    nc.scalar.activation(
                        out=hg, in_=ps[:, :],
                        func=mybir.ActivationFunctionType.Relu)
                    h3 = hg.rearrange("p (t b) -> p t b", b=B)
                    nc.vector.reduce_sum(
                        out=s1a[:, g * GROUP:(g + 1) * GROUP], in_=h3,
                        axis=mybir.AxisListType.X)
                    sq = wpool.tile([128, GB], f32, tag="sq")
                    nc.vector.tensor_mul(sq[:, :], hg, hg)
                    nc.vector.reduce_sum(
                        out=s2a[:, g * GROUP:(g + 1) * GROUP],
                        in_=sq.rearrange("p (t b) -> p t b", b=B),
                        axis=mybir.AxisListType.X)

            # ---------------- Phase 1b: batched BN stats -> scale/shift ------
            # var = (32*S2 - S1^2)/1024; std = sqrt(var) = sqrt(q)/32
            with tc.tile_pool(name="stat", bufs=1) as stat:
                t1 = stat.tile([128, N_TILES], f32)
                nc.vector.tensor_mul(t1[:, :], s1a[:, :], s1a[:, :])
                q = stat.tile([128, N_TILES], f32)
                # q = 32*S2 - S1^2
                nc.vector.scalar_tensor_tensor(
                    out=q[:, :], in0=s2a[:, :], scalar=float(B),
                    in1=t1[:, :], op0=mybir.AluOpType.mult,
                    op1=mybir.AluOpType.subtract)
                # std = sqrt(q/B^2)
                nc.scalar.activation(
                    out=q[:, :], in_=q[:, :],
                    func=mybir.ActivationFunctionType.Sqrt,
                    scale=float(1.0 / (B * B)))
                # inv = 1/(std + eps)
                nc.vector.tensor_scalar_add(
                    out=q[:, :], in0=q[:, :], scalar1=float(BN_EPS))
                nc.vector.reciprocal(out=q[:, :], in_=q[:, :])
                nc.vector.tensor_mul(scl[:, :], q[:, :], gam[:, :])
                # shift = beta - (S1/B)*scale
                nc.vector.tensor_mul(t1[:, :], s1a[:, :], scl[:, :])
                nc.vector.scalar_tensor_tensor(
                    out=sft[:, :], in0=t1[:, :], scalar=float(-1.0 / B),
                    in1=bet[:, :], op0=mybir.AluOpType.mult,
                    op1=mybir.AluOpType.add)

            # ---------------- Phase 1c: normalize + transpose + store --------
            with (
                tc.tile_pool(name="npool", bufs=3) as npool,
                tc.tile_pool(name="pt", bufs=2, space="PSUM") as psum_t,
            ):
                for g in range(N_GROUPS):
                    hg3 = h_all[:, g * GB:(g + 1) * GB] \
                        .rearrange("p (t b) -> p t b", b=B)
                    tmp = npool.tile([128, GB], f32, tag="tmp")
                    nc.vector.tensor_mul(
                        tmp.rearrange("p (t b) -> p t b", b=B), hg3,
                        scl[:, g * GROUP:(g + 1) * GROUP]
                        .to_broadcast([128, GROUP, B]))
                    kern = npool.tile([128, GB], K_DT, tag="kern")
                    nc.vector.tensor_tensor(
                        out=kern.rearrange("p (t b) -> p t b", b=B),
                        in0=tmp.rearrange("p (t b) -> p t b", b=B),
                        in1=sft[:, g * GROUP:(g + 1) * GROUP]
                        .to_broadcast([128, GROUP, B]),
                        op=mybir.AluOpType.add)
                    pt = psum_t.tile([B, GC], K_DT, tag="pt")
                    for j in range(GROUP):
                        nc.tensor.transpose(
                            pt[:, j * 128:(j + 1) * 128],
                            kern[:, j * B:(j + 1) * B],
                            ident[:, :],
                        )
                    ktr = npool.tile([B, GC], K_DT, tag="ktr")
                    nc.any.tensor_copy(out=ktr[:, :], in_=pt[:, :])
                    nc.sync.dma_start(
                        out=cc_in[:, g * GC:(g + 1) * GC], in_=ktr[:, :])

            # ---------------- Phase 2 prep (overlaps the collective) ---------
            with (
                tc.tile_pool(name="conv", bufs=1) as conv,
                tc.tile_pool(name="po", bufs=2, space="PSUM") as psum_o,
            ):
                x_all = conv.tile([128, B_L, PIX], f32)
                nc.sync.dma_start(
                    out=x_all[:, :, :],
                    in_=x_in.rearrange("b p j -> p b j"))
                xp_all = conv.tile([128, B_L, PAD * PAD], K_DT)
                nc.vector.memset(xp_all[:, :, :], 0.0)
                nc.vector.tensor_copy(
                    out=xp_all.rearrange("p b (r c) -> p b r c", c=PAD)
                    [:, :, 1:H + 1, 1:W + 1],
                    in_=x_all.rearrange("p b (r c) -> p b r c", c=W),
                )

                # ---------------- AllToAll: batch redistribute ---------------
                nc.gpsimd.collective_compute(
                    "AllToAll",
                    mybir.AluOpType.bypass,
                    replica_groups=[list(range(N_CORES))],
                    ins=[cc_in[:, :]],
                    outs=[cc_out[:, :]],
                )

                # ---------------- Phase 2: per-sample conv + residual --------
                hk_all = conv.tile([128, B_L, C * KH * KW], K_DT)
                for i in range(B_L):
                    src = bass.AP(
                        tensor=cc_out[:, :].tensor,
                        offset=i * NB_L,
                        ap=[[B_L * NB_L, N_CORES], [C * KH * KW, S_L],
                            [1, C * KH * KW]],
                    )
                    nc.sync.dma_start(out=hk_all[:, i, :], in_=src)
                out_all = conv.tile([128, B_L, PIX], f32)
                for i in range(B_L):
                    po = psum_o.tile([128, PIX], f32, tag="po")
                    hk9 = hk_all[:, i, :].rearrange("p (f n) -> p n f", n=KH * KW)
                    xp3 = xp_all[:, i, :].rearrange("p (r c) -> p r c", c=PAD)
                    for hh in range(2):
                        for tap in range(KH * KW):
                            u, v = tap // KW, tap % KW
                            r0 = hh * 16 + u
                            nc.tensor.matmul(
                                po[:, hh * 512:(hh + 1) * 512],
                                hk9[:, tap, :],
                                xp3[:, r0:r0 + 16, v:v + W],
                                start=(tap == 0),
                                stop=(tap == KH * KW - 1),
                            )
                    # out = conv + x (+ b)
                    nc.vector.tensor_add(
                        out=out_all[:, i, :], in0=po[:, :], in1=x_all[:, i, :])
                    if not b_zero:
                        nc.scalar.add(
                            out=out_all[:, i, :], in_=out_all[:, i, :],
                            add=bv[:, 0:1])
                nc.sync.dma_start(
                    out=out_p.rearrange("b p j -> p b j"),
                    in_=out_all[:, :, :])

    nc.compile()
    return nc


def _make_in_maps(x, z, dense_w, dense_b, gamma, beta, b):
    wnp = _np_of(W_DT)
    f32 = np.float32
    # zT[p, k*B + bb] = z[bb, 128k + p]
    zr = np.ascontiguousarray(z.T.astype(f32)).reshape(2, 128, B)
    zT = np.concatenate([zr[0], zr[1]], axis=1).astype(wnp)
    bvec = np.asarray(b, dtype=f32).reshape(C, 1)
    in_maps = []
    for c in range(N_CORES):
        sl = slice(c * NB_L, (c + 1) * NB_L)
        ws = dense_w[:, sl]
        w_host = np.ascontiguousarray(
            np.concatenate([ws[:128, :], ws[128:, :]], axis=1)).astype(wnp)
        in_maps.append({
            "w": w_host,
            "zT": zT,
            "db_r": np.ascontiguousarray(
                np.asarray(dense_b, dtype=f32)[sl].reshape(N_TILES, 128).T),
            "gamma_r": np.ascontiguousarray(
                np.asarray(gamma, dtype=f32)[sl].reshape(N_TILES, 128).T),
            "beta_r": np.ascontiguousarray(
                np.asarray(beta, dtype=f32)[sl].reshape(N_TILES, 128).T),
            "x": np.ascontiguousarray(
                np.asarray(x, dtype=f32)[c * B_L:(c + 1) * B_L]
                .reshape(B_L, C, PIX)),
            "bvec": bvec,
        })
    return in_maps


def kernel(x, z, dense_w, dense_b, gamma, beta, b):
    key = (bool(np.all(np.asarray(dense_b) == 0)),
           bool(np.all(np.asarray(b) == 0)))
    if key not in _CACHE:
        _CACHE[key] = _build_nc(*key)
        _CACHE["nc"] = _CACHE[key]
    nc = _CACHE[key]
    in_maps = _make_in_maps(x, z, dense_w, dense_b, gamma, beta, b)
    res = run_bass_kernel_spmd(nc, in_maps, list(range(N_CORES)))
    out = np.concatenate(
        [res.results[c]["out"].reshape(B_L, C, H, W) for c in range(N_CORES)],
        axis=0,
    )
    return out.astype(np.float32)
